# revision 13
# baseline (speedup 1.0000x reference)
import sys

sys.path.insert(0, "/opt/trn_rl_repo")
from contextlib import ExitStack

import numpy as np
import ml_dtypes

import concourse.bass as bass
import concourse.tile as tile
from concourse import bacc, mybir
from concourse.bass_utils import run_bass_kernel_spmd

NCORES = 8
N_NODES, N_GRAPHS, GL = 8192, 64, 128
N_EDGES, HEADS = 49152, 8
GPC = N_GRAPHS // NCORES          # graphs per core = 8
NPC = GPC * GL                    # nodes per core = 1024
D1 = 2048
D4H = 1024
HID = 1024
G4 = 4096
T = GL

bf16 = mybir.dt.float16  # fp16: 11-bit mantissa needed for saturated attention logits
f32 = mybir.dt.float32
ALU = mybir.AluOpType
ACTF = mybir.ActivationFunctionType

_NC_CACHE = {}


def _leaky_softmax_alpha_T(nc, sp, psm, tag, identf, esed_nm, lnc_sb, g, h, identb,
                           scale2):
    """Build alpha^T [src, dst] bf16 for (graph g, head h)."""
    pse = psm.tile([1, 128], f32, name=f"pseT{tag}", tag="ps_small")
    nc.tensor.transpose(pse[:], esed_nm[:, g, h:h + 1], identf[:])
    esT = sp.tile([1, 128], f32, name=f"esT{tag}", tag="esT")
    nc.vector.tensor_copy(esT[:], pse[:])
    esb = sp.tile([128, 128], f32, name=f"esb{tag}", tag="esb")
    nc.gpsimd.partition_broadcast(esb[:], esT[0:1, :])
    nc.scalar.activation(esb[:], esb[:], ACTF.Identity,
                         bias=esed_nm[:, g, 8 + h:9 + h])
    nc.vector.scalar_tensor_tensor(esb[:], esb[:], 0.2, esb[:], op0=ALU.mult,
                                   op1=ALU.max)
    nc.vector.tensor_tensor(esb[:], esb[:], lnc_sb[:, g, :], op=ALU.add)
    nm = sp.tile([128, 1], f32, name=f"nm{tag}", tag="negmax")
    nc.vector.tensor_reduce(nm[:], esb[:], axis=mybir.AxisListType.X, op=ALU.max,
                            negate=True)
    z_t = sp.tile([128, 1], f32, name=f"z{tag}", tag="z")
    nc.scalar.activation(esb[:], esb[:], ACTF.Exp, bias=nm[:, 0:1],
                         accum_out=z_t[:])
    zi = sp.tile([128, 1], f32, name=f"zi{tag}", tag="zi")
    nc.vector.reciprocal(zi[:], z_t[:])
    al = sp.tile([128, 128], bf16, name=f"al{tag}", tag="al")
    if scale2 is None:
        nc.vector.tensor_scalar_mul(al[:], esb[:], zi[:, 0:1])
    else:
        nc.vector.tensor_scalar(al[:], esb[:], zi[:, 0:1], scale2,
                                op0=ALU.mult, op1=ALU.mult)
    psa = psm.tile([128, 128], bf16, name=f"psa{tag}", tag="ps_small")
    nc.tensor.transpose(psa[:], al[:], identb[:])
    alT = sp.tile([128, 128], bf16, name=f"alT{tag}", tag="alT")
    nc.vector.tensor_copy(alT[:], psa[:])
    return alT


def _elu_ln(nc, sp, lay, g, src_ap, F, xd_prev, xpool):
    """xg = LN(elu(src)) (+ residual from DRAM).  src_ap: [128, F] f32."""
    t1 = sp.tile([128, F], bf16, name=f"eA{lay}", tag="eluA")
    nc.vector.tensor_scalar_min(t1[:], src_ap, 0.0)
    nc.scalar.activation(t1[:], t1[:], ACTF.Exp)
    s = sp.tile([128, F], f32, name=f"eS{lay}", tag="eluS")
    ssum = sp.tile([128, 1], f32, name=f"ss{lay}", tag="ssum")
    nc.vector.scalar_tensor_tensor(s[:], src_ap, 0.0, t1[:], op0=ALU.max,
                                   op1=ALU.add, accum_out=ssum[:])
    negmu = sp.tile([128, 1], f32, name=f"nm{lay}", tag="negmu")
    nc.vector.tensor_scalar_mul(negmu[:], ssum[:], -1.0 / F)
    vsum = sp.tile([128, 1], f32, name=f"vs{lay}", tag="vsum")
    nc.scalar.activation(t1[:], s[:], ACTF.Square, bias=negmu[:, 0:1],
                         accum_out=vsum[:])  # t1 dead; bf16 out ok (accum is f32)
    vs2 = sp.tile([128, 1], f32, name=f"vs2{lay}", tag="vsum2")
    nc.vector.tensor_scalar_add(vs2[:], vsum[:], 1e-5 * F)
    sd = sp.tile([128, 1], f32, name=f"sd{lay}", tag="sd")
    nc.scalar.activation(sd[:], vs2[:], ACTF.Sqrt, scale=1.0 / F)
    rinv = sp.tile([128, 1], f32, name=f"ri{lay}", tag="rinv")
    nc.vector.reciprocal(rinv[:], sd[:])
    b2 = sp.tile([128, 1], f32, name=f"b2{lay}", tag="b2")
    nc.vector.tensor_tensor(b2[:], negmu[:], rinv[:], op=ALU.mult)
    xg = xpool.tile([128, F], bf16, name=f"X{lay}_{g}", tag="X", bufs=3)
    if xd_prev is None:
        nc.scalar.activation(xg[:], s[:], ACTF.Identity, bias=b2[:, 0:1],
                             scale=rinv[:, 0:1])
    else:
        xr = sp.tile([128, F], bf16, name=f"xr{lay}", tag="Xres", bufs=2)
        nc.sync.dma_start(xr[:], xd_prev[g])
        nc.scalar.activation(t1[:], s[:], ACTF.Identity, bias=b2[:, 0:1],
                             scale=rinv[:, 0:1])
        nc.vector.tensor_tensor(xg[:], t1[:], xr[:], op=ALU.add)
    return xg


def _gat_layer(nc, tc, ctx, lay, XT_in, KC, wname, lnc_sb, identb, identf,
               xd_prev, xd_out, xpool, tpool, din, dbg_out):
    d = 256
    Fo = D1
    w_dram = din[wname]

    wk = ctx.enter_context(tc.tile_pool(name=f"wk{lay}", bufs=KC))
    hp = ctx.enter_context(tc.tile_pool(name=f"hp{lay}", bufs=2))
    sp = ctx.enter_context(tc.tile_pool(name=f"sp{lay}", bufs=1))
    sps = ctx.enter_context(tc.tile_pool(name=f"sps{lay}", bufs=2))
    pmm = ctx.enter_context(tc.tile_pool(name=f"pmm{lay}", bufs=2, space="PSUM"))
    psm = ctx.enter_context(tc.tile_pool(name=f"psm{lay}", bufs=2, space="PSUM"))
    pout = ctx.enter_context(tc.tile_pool(name=f"pout{lay}", bufs=1, space="PSUM"))

    wsrc = w_dram.rearrange("(kc p) f -> p kc f", p=128) if KC > 3 else None

    # --- es/ed pass (tiny weights) ---
    wesd = sps.tile([128, KC, 16], bf16, name=f"wesd{lay}", tag="wesd", bufs=1)
    if KC == 3:
        for k in range(KC):
            nc.sync.dma_start(wesd[:, k, :],
                              w_dram[k * 128:(k + 1) * 128, Fo:Fo + 16])
    else:
        nc.sync.dma_start(wesd[:], wsrc[:, :, Fo:Fo + 16])
    esed_nm = sps.tile([128, 8, 16], f32, name=f"esed{lay}", tag="esed", bufs=1)
    for m in range(8):
        pse = psm.tile([128, 128], f32, name=f"psE{lay}", tag="ps_small")
        for k in range(KC):
            nc.tensor.matmul(pse[:, 0:16], XT_in[:, k, m * 128:(m + 1) * 128],
                             wesd[:, k, :], start=(k == 0), stop=(k == KC - 1))
        nc.vector.tensor_copy(esed_nm[:, m, :], pse[:, 0:16])
    # --- weights resident ---
    w_sb = []
    for k in range(KC):
        wt = wk.tile([128, Fo], bf16, name=f"w{lay}_{k}", tag="wblk")
        nc.sync.dma_start(wt[:], w_dram[k * 128:(k + 1) * 128, 0:Fo])
        w_sb.append(wt)

    # --- per graph: H then attention then elu/LN ---
    XT_next = tpool.tile([128, 16, NPC], bf16, name=f"XT{lay + 1}", tag="XT",
                         bufs=2)
    for g in range(8):
        ht = hp.tile([128, Fo], bf16, name=f"H{lay}", tag="H")
        for n in range(4):
            ps = pmm.tile([128, 512], f32, name=f"psH{lay}", tag="ps_mm")
            for k in range(KC):
                nc.tensor.matmul(ps[:], XT_in[:, k, g * 128:(g + 1) * 128],
                                 w_sb[k][:, n * 512:(n + 1) * 512],
                                 start=(k == 0), stop=(k == KC - 1))
            if n % 2 == 0:
                nc.scalar.copy(ht[:, n * 512:(n + 1) * 512], ps[:])
            else:
                nc.vector.tensor_copy(ht[:, n * 512:(n + 1) * 512], ps[:])
        pB = pout.tile([128, Fo], f32, name=f"psO{lay}", tag="ps_out")
        for h in range(8):
            alT = _leaky_softmax_alpha_T(nc, sps, psm, f"{lay}_{g}_{h}", identf,
                                         esed_nm, lnc_sb, g, h, identb, None)
            nc.tensor.matmul(pB[:, h * d:(h + 1) * d], alT[:],
                             ht[:, h * d:(h + 1) * d], start=True, stop=True)
        xg = _elu_ln(nc, sp, lay, g, pB[:], Fo, xd_prev, xpool)
        if xd_out is not None:
            nc.sync.dma_start(xd_out[g], xg[:])
        for k in range(16):
            pst = psm.tile([128, 128], bf16, name=f"psX{lay}", tag="ps_small")
            nc.tensor.transpose(pst[:], xg[:, k * 128:(k + 1) * 128], identb[:])
            if k % 2 == 0:
                nc.scalar.copy(XT_next[:, k, g * 128:(g + 1) * 128], pst[:])
            else:
                nc.vector.tensor_copy(XT_next[:, k, g * 128:(g + 1) * 128], pst[:])
        if dbg_out is not None:
            dt_ = sp.tile([128, Fo], f32, name=f"dbg{lay}", tag="eluS")
            nc.vector.tensor_copy(dt_[:], xg[:])
            nc.sync.dma_start(dbg_out[g], dt_[:])
    return XT_next


def _gat_layer4(nc, tc, ctx, XT4, lnc_sb, identb, identf, xpool, tpool, din,
                dbg_out):
    d = D4H
    Fo = HEADS * D4H
    w_dram = din["w4"]
    sp = ctx.enter_context(tc.tile_pool(name="sp_l4", bufs=1))
    sps = ctx.enter_context(tc.tile_pool(name="sps_l4", bufs=2))
    hp = ctx.enter_context(tc.tile_pool(name="hp_l4", bufs=2))
    wkp = ctx.enter_context(tc.tile_pool(name="wk_l4", bufs=4))
    pmm = ctx.enter_context(tc.tile_pool(name="pmm_l4", bufs=2, space="PSUM"))
    psm = ctx.enter_context(tc.tile_pool(name="psm_l4", bufs=2, space="PSUM"))
    pout = ctx.enter_context(tc.tile_pool(name="pout_l4", bufs=2, space="PSUM"))

    wsrc = w_dram.rearrange("(kc p) f -> p kc f", p=128)
    wesd = sps.tile([128, 16, 16], bf16, name="wesd4", tag="wesd", bufs=1)
    nc.sync.dma_start(wesd[:], wsrc[:, :, Fo:Fo + 16])
    esed_nm = sps.tile([128, 8, 16], f32, name="esed4", tag="esed", bufs=1)
    for m in range(8):
        pse = psm.tile([128, 128], f32, name="psE4", tag="ps_small")
        for k in range(16):
            nc.tensor.matmul(pse[:, 0:16], XT4[:, k, m * 128:(m + 1) * 128],
                             wesd[:, k, :], start=(k == 0), stop=(k == 15))
        nc.vector.tensor_copy(esed_nm[:, m, :], pse[:, 0:16])
    X4T = tpool.tile([128, 8, NPC], bf16, name="X4T", tag="XT", bufs=2)
    for g in range(8):
        pB = pout.tile([128, D4H], f32, name="psO4", tag="ps_out4")
        for h in range(HEADS):
            h4 = hp.tile([128, d], bf16, name="h4", tag="h4")
            for n in range(2):
                ps = pmm.tile([128, 512], f32, name="psH4", tag="ps_mm")
                for k in range(16):
                    wkb = wkp.tile([128, 512], bf16, name="w4kb", tag="w4kb")
                    nc.sync.dma_start(
                        wkb[:], wsrc[:, k, h * d + n * 512:h * d + (n + 1) * 512])
                    nc.tensor.matmul(ps[:], XT4[:, k, g * 128:(g + 1) * 128],
                                     wkb[:], start=(k == 0), stop=(k == 15))
                if n % 2 == 0:
                    nc.scalar.copy(h4[:, n * 512:(n + 1) * 512], ps[:])
                else:
                    nc.vector.tensor_copy(h4[:, n * 512:(n + 1) * 512], ps[:])
            alT = _leaky_softmax_alpha_T(nc, sps, psm, f"4_{g}_{h}", identf,
                                         esed_nm, lnc_sb, g, h, identb, 0.125)
            nc.tensor.matmul(pB[:, 0:512], alT[:], h4[:, 0:512],
                             start=(h == 0), stop=(h == 7))
            nc.tensor.matmul(pB[:, 512:1024], alT[:], h4[:, 512:1024],
                             start=(h == 0), stop=(h == 7))
        xg = _elu_ln(nc, sp, 4, g, pB[:], D4H, None, xpool)
        for k in range(8):
            pst = psm.tile([128, 128], bf16, name="psXT4", tag="ps_small")
            nc.tensor.transpose(pst[:], xg[:, k * 128:(k + 1) * 128], identb[:])
            if k % 2 == 0:
                nc.scalar.copy(X4T[:, k, g * 128:(g + 1) * 128], pst[:])
            else:
                nc.vector.tensor_copy(X4T[:, k, g * 128:(g + 1) * 128], pst[:])
        if dbg_out is not None:
            dt_ = sp.tile([128, D4H], f32, name="dbg4", tag="eluS")
            nc.vector.tensor_copy(dt_[:], xg[:])
            nc.sync.dma_start(dbg_out[g], dt_[:])
    return X4T


def _xw_phase(nc, tc, ctx, din, X4T, identb, agin_f, agin_b, agout_f, agout_b,
              dbg_xw):
    sp4 = ctx.enter_context(tc.tile_pool(name="sp_xw", bufs=4))
    ps4 = ctx.enter_context(tc.tile_pool(name="ps_xw", bufs=4, space="PSUM"))
    bg_sb = sp4.tile([128, 64], f32, name="bg_sb", tag="bg", bufs=1)
    nc.sync.dma_start(bg_sb[:], din["bg2"][:])
    wih = din["wihT2"].rearrange("(kc p) (dd m) -> p kc dd m", p=128, dd=2)
    # rhs columns in t-major order: col' = t*8+g  <- node g*128+t
    X4T_tmaj = X4T[:].rearrange("p k (g t) -> p k t g", g=8)
    for di, agin in enumerate((agin_f, agin_b)):
        for m in range(32):
            wm = sp4.tile([128, 8, 128], bf16, name="wm", tag="wm")
            nc.sync.dma_start(wm[:], wih[:, :, di, m * 128:(m + 1) * 128])
            for n2 in range(2):
                ps = ps4.tile([128, 512], f32, name="psxw", tag="ps_mm")
                rhs = X4T_tmaj[:, :, n2 * 64:(n2 + 1) * 64, :]
                for k in range(8):
                    nc.tensor.matmul(ps[:],
                                     wm[:, k, :],
                                     rhs[:, k, :, :],
                                     start=(k == 0), stop=(k == 7))
                xwe = sp4.tile([128, 512], bf16, name="xwe", tag="xwe")
                psv = ps.rearrange("p (t g) -> p t g", g=8)
                src_ap = psv[:, ::-1, :] if di == 1 else psv[:, :, :]
                nc.scalar.activation(xwe.rearrange("p (t g) -> p t g", g=8),
                                     src_ap, ACTF.Identity,
                                     bias=bg_sb[:, di * 32 + m:di * 32 + m + 1])
                # NOTE backward: block of t written reversed within its half;
                # halves must also swap to get a full reversal
                dn2 = (1 - n2) if di == 1 else n2
                nc.sync.dma_start(
                    agin[m * 128:(m + 1) * 128, dn2 * 512:(dn2 + 1) * 512],
                    xwe[:])
    nc.gpsimd.collective_compute(
        "AllGather", ALU.bypass, replica_groups=[list(range(NCORES))],
        ins=[agin_f.opt()], outs=[agout_f.opt()])
    nc.gpsimd.collective_compute(
        "AllGather", ALU.bypass, replica_groups=[list(range(NCORES))],
        ins=[agin_b.opt()], outs=[agout_b.opt()])
    if dbg_xw is not None:
        for di, agin in enumerate((agin_f, agin_b)):
            for m in range(32):
                ts_ = sp4.tile([128, NPC], bf16, name="dxwb", tag="dxwb")
                nc.sync.dma_start(ts_[:], agin[m * 128:(m + 1) * 128, :])
                t_ = sp4.tile([128, NPC], f32, name="dxw", tag="dxw")
                nc.vector.tensor_copy(t_[:], ts_[:])
                nc.sync.dma_start(dbg_xw[di, m * 128:(m + 1) * 128, :], t_[:])


def _lstm(nc, tc, ctx, din, agout_f, agout_b, hb_in, dbg_h):
    TB = 8
    sp = ctx.enter_context(tc.tile_pool(name="sp_ls", bufs=2))
    gp = ctx.enter_context(tc.tile_pool(name="gp_ls", bufs=2))
    xwp = ctx.enter_context(tc.tile_pool(name="xw_ls", bufs=2))
    pg = ctx.enter_context(tc.tile_pool(name="pg_ls", bufs=2, space="PSUM"))

    whh = ctx.enter_context(tc.tile_pool(name="whh_p", bufs=1)) \
        .tile([128, 8, G4], bf16, name="whh_sb")
    nc.sync.dma_start(whh[:], din["whhT"].rearrange("(kc p) m -> p kc m", p=128))

    pid = nc.gpsimd.partition_id()
    is_lo = nc.gpsimd.compute_val(pid < 4)
    is_hi = nc.gpsimd.compute_val(pid >= 4)

    hcur = sp.tile([128, 8, 64], bf16, name="h0", tag="h")
    ccur = sp.tile([128, 8, 64], f32, name="c0", tag="c")
    nc.vector.memset(hcur[:], 0.0)
    nc.vector.memset(ccur[:], 0.0)

    # agout: [r, 4096, 1024] with node dim t-major (t*8+g_local)
    agf = agout_f[:].rearrange("r (mc p) n -> p mc r n", p=128)
    agb = agout_b[:].rearrange("r (mc p) n -> p mc r n", p=128)

    for tb in range(T // TB):
        xwb = xwp.tile([128, 32, 8, TB, 8], bf16, name="xwb", tag="xwb")
        for m in range(32):
            nc.gpsimd.dma_start(
                xwb[:, m, :, :, :],
                agf[:, m, :, tb * TB * 8:(tb + 1) * TB * 8]
                .rearrange("p r (t g) -> p r t g", g=8), cond=is_lo)
            nc.gpsimd.dma_start(
                xwb[:, m, :, :, :],
                agb[:, m, :, tb * TB * 8:(tb + 1) * TB * 8]
                .rearrange("p r (t g) -> p r t g", g=8), cond=is_hi)
        for tl in range(TB):
            pgt = [pg.tile([128, 8, 64], f32, name=f"pg{G}", tag=f"ps_g{G}")
                   for G in range(4)]
            for m32 in range(32):
                G, u = divmod(m32, 8)
                for k in range(8):
                    nc.tensor.matmul(pgt[G][:, u, :],
                                     whh[:, k, m32 * 128:(m32 + 1) * 128],
                                     hcur[:, k, :], start=(k == 0), stop=(k == 7))
            pre = [gp.tile([128, 8, 8, 8], f32, name=f"pre{G}", tag=f"pre{G}")
                   for G in range(4)]
            for G in range(4):
                nc.vector.tensor_tensor(
                    pre[G][:], pgt[G].rearrange("p u (r g) -> p u r g", g=8),
                    xwb[:, G * 8:(G + 1) * 8, :, tl, :], op=ALU.add)
            si = gp.tile([128, 8, 64], f32, name="si", tag="si")
            sf = gp.tile([128, 8, 64], f32, name="sf", tag="sf")
            tg = gp.tile([128, 8, 64], f32, name="tg", tag="tg")
            so = gp.tile([128, 8, 64], f32, name="so", tag="so")
            nc.scalar.activation(si[:].rearrange("p u (r g) -> p u r g", g=8),
                                 pre[0][:], ACTF.Sigmoid)
            nc.scalar.activation(sf[:].rearrange("p u (r g) -> p u r g", g=8),
                                 pre[1][:], ACTF.Sigmoid)
            nc.scalar.activation(tg[:].rearrange("p u (r g) -> p u r g", g=8),
                                 pre[2][:], ACTF.Tanh)
            nc.scalar.activation(so[:].rearrange("p u (r g) -> p u r g", g=8),
                                 pre[3][:], ACTF.Sigmoid)
            m1 = gp.tile([128, 8, 64], f32, name="m1", tag="m1")
            nc.vector.tensor_tensor(m1[:], sf[:], ccur[:], op=ALU.mult)
            m2 = gp.tile([128, 8, 64], f32, name="m2", tag="m2")
            nc.vector.tensor_tensor(m2[:], si[:], tg[:], op=ALU.mult)
            cnew = sp.tile([128, 8, 64], f32, name="cn", tag="c")
            nc.vector.tensor_tensor(cnew[:], m1[:], m2[:], op=ALU.add)
            tct = gp.tile([128, 8, 64], f32, name="tct", tag="tct")
            nc.scalar.activation(tct[:], cnew[:], ACTF.Tanh)
            hnew = sp.tile([128, 8, 64], bf16, name="hn", tag="h")
            nc.vector.tensor_tensor(hnew[:], so[:], tct[:], op=ALU.mult)
            hcur, ccur = hnew, cnew

    nc.sync.dma_start(hb_in[:], hcur[:])
    if dbg_h is not None:
        t_ = sp.tile([128, 8, 64], f32, name="dbgh", tag="dbgh")
        nc.vector.tensor_copy(t_[:], hcur[:])
        nc.sync.dma_start(dbg_h.rearrange("p (u b) -> p u b", b=64), t_[:])


def _fc(nc, tc, ctx, din, hb_out, out):
    sp = ctx.enter_context(tc.tile_pool(name="sp_fc", bufs=1))
    ps = ctx.enter_context(tc.tile_pool(name="ps_fc", bufs=2, space="PSUM"))
    hF = sp.tile([128, 8, 64], bf16, name="hF")
    hB = sp.tile([128, 8, 64], bf16, name="hB")
    nc.sync.dma_start(hF[:], hb_out[0])
    nc.sync.dma_start(hB[:], hb_out[4])
    w1 = sp.tile([128, 16, 512], bf16, name="fc1w_sb")
    nc.sync.dma_start(w1[:], din["fc1w"].rearrange("(kc p) m -> p kc m", p=128))
    b1 = sp.tile([128, 4], f32, name="fc1b_sb")
    nc.sync.dma_start(b1[:], din["fc1b"][:])
    w2 = sp.tile([128, 4, 104], bf16, name="fc2w_sb")
    nc.sync.dma_start(w2[:], din["fc2w"].rearrange("(kc p) m -> p kc m", p=128))
    b2 = sp.tile([104, 1], f32, name="fc2b_sb")
    nc.sync.dma_start(b2[:], din["fc2b"][:])

    r1 = sp.tile([128, 4, 64], bf16, name="r1")
    for m in range(4):
        p1 = ps.tile([128, 64], f32, name="psf1", tag="f1")
        for k in range(16):
            rhs = hF[:, k, :] if k < 8 else hB[:, k - 8, :]
            nc.tensor.matmul(p1[:], w1[:, k, m * 128:(m + 1) * 128], rhs,
                             start=(k == 0), stop=(k == 15))
        nc.scalar.activation(r1[:, m, :], p1[:], ACTF.Relu, bias=b1[:, m:m + 1])
    p2 = ps.tile([104, 64], f32, name="psf2", tag="f2")
    for k in range(4):
        nc.tensor.matmul(p2[:], w2[:, k, :], r1[:, k, :],
                         start=(k == 0), stop=(k == 3))
    o = sp.tile([104, 64], f32, name="o_sb")
    nc.scalar.activation(o[:], p2[:], ACTF.Identity, bias=b2[:, 0:1])
    nc.sync.dma_start(out[:], o[:])


def _build(dbg=False):
    nc = bacc.Bacc(None, target_bir_lowering=False, num_devices=NCORES)

    din = {}

    def dt_in(name, shape, dt):
        din[name] = nc.dram_tensor(name, shape, dt, kind="ExternalInput")

    dt_in("xT", [384, NPC], bf16)
    dt_in("w1", [384, D1 + 16], bf16)
    dt_in("w2", [D1, D1 + 16], bf16)
    dt_in("w3", [D1, D1 + 16], bf16)
    dt_in("w4", [D1, HEADS * D4H + 16], bf16)
    dt_in("lnc", [128, GPC, 128], f32)
    dt_in("identb", [128, 128], bf16)
    dt_in("identf", [128, 128], f32)
    dt_in("wihT2", [HID, 2 * G4], bf16)
    dt_in("bg2", [128, 64], f32)
    dt_in("whhT", [HID, G4], bf16)
    dt_in("fc1w", [D1, 512], bf16)
    dt_in("fc1b", [128, 4], f32)
    dt_in("fc2w", [512, 104], bf16)
    dt_in("fc2b", [104, 1], f32)

    out = nc.dram_tensor("out", [104, 64], f32, kind="ExternalOutput")
    dbg_x = dbg_xw = dbg_h = None
    if dbg:
        dbg_x = {l: nc.dram_tensor(f"dbg_x{l}", [8, 128, D1 if l < 4 else D4H],
                                   f32, kind="ExternalOutput")
                 for l in (1, 2, 3, 4)}
        dbg_xw = nc.dram_tensor("dbg_xw", [2, G4, NPC], f32, kind="ExternalOutput")
        dbg_h = nc.dram_tensor("dbg_h", [128, 512], f32, kind="ExternalOutput")

    with tile.TileContext(nc) as tc:
        with ExitStack() as octx:
            const = octx.enter_context(tc.tile_pool(name="const", bufs=1))
            identb = const.tile([128, 128], bf16, name="identb_sb")
            nc.sync.dma_start(identb[:], din["identb"][:])
            identf = const.tile([128, 128], f32, name="identf_sb")
            nc.sync.dma_start(identf[:], din["identf"][:])
            lnc_sb = const.tile([128, GPC, 128], f32, name="lnc_sb")
            nc.sync.dma_start(lnc_sb[:], din["lnc"][:])

            dram = octx.enter_context(tc.tile_pool(name="dram", bufs=1,
                                                   space="DRAM"))
            agin_f = dram.tile([G4, NPC], bf16, name="agin_f")
            agin_b = dram.tile([G4, NPC], bf16, name="agin_b")
            agout_f = dram.tile([NCORES, G4, NPC], bf16, name="agout_f",
                                addr_space="Shared")
            agout_b = dram.tile([NCORES, G4, NPC], bf16, name="agout_b",
                                addr_space="Shared")
            hb_in = dram.tile([128, 8, 64], bf16, name="hb_in")
            xd1 = dram.tile([8, 128, D1], bf16, name="xd1")
            xd2 = dram.tile([8, 128, D1], bf16, name="xd2")
            hb_out = dram.tile([NCORES, 128, 8, 64], bf16, name="hb_out",
                               addr_space="Shared")

            xfer_cm = tc.tile_pool(name="xfer", bufs=1)
            xfer = xfer_cm.__enter__()
            with ExitStack() as c0:
                l1in = c0.enter_context(tc.tile_pool(name="l1in", bufs=1))
                XT1 = l1in.tile([128, 3, NPC], bf16, name="XT1")
                for k in range(3):
                    nc.sync.dma_start(XT1[:, k, :],
                                      din["xT"][k * 128:(k + 1) * 128, :])
                XT2 = _gat_layer(nc, tc, c0, 1, XT1, 3, "w1", lnc_sb, identb,
                                 identf, None, xd1, xfer, xfer, din,
                                 dbg_x[1] if dbg else None)
            with ExitStack() as c1:
                XT3 = _gat_layer(nc, tc, c1, 2, XT2, 16, "w2", lnc_sb, identb,
                                 identf, xd1, xd2, xfer, xfer, din,
                                 dbg_x[2] if dbg else None)
            with ExitStack() as c2:
                XT4 = _gat_layer(nc, tc, c2, 3, XT3, 16, "w3", lnc_sb, identb,
                                 identf, xd2, None, xfer, xfer, din,
                                 dbg_x[3] if dbg else None)
            with ExitStack() as c3:
                X4T = _gat_layer4(nc, tc, c3, XT4, lnc_sb, identb, identf, xfer,
                                  xfer, din, dbg_x[4] if dbg else None)
            with ExitStack() as c4:
                _xw_phase(nc, tc, c4, din, X4T, identb, agin_f, agin_b,
                          agout_f, agout_b, dbg_xw)
            xfer_cm.__exit__(None, None, None)
            with ExitStack() as c5:
                _lstm(nc, tc, c5, din, agout_f, agout_b, hb_in, dbg_h)
            with ExitStack() as c6:
                nc.gpsimd.collective_compute(
                    "AllGather", ALU.bypass,
                    replica_groups=[list(range(NCORES))],
                    ins=[hb_in.opt()], outs=[hb_out.opt()])
                _fc(nc, tc, c6, din, hb_out, out)

    nc.finalize()
    return nc


# ---------------------------------------------------------------- host side

def _b16(a):
    return np.asarray(a, np.float32).astype(np.float16)


def _prep_inputs(x, src, dst, params):
    p = {k: np.asarray(v, np.float32) for k, v in params.items()}
    x = np.asarray(x, np.float32)
    src = np.asarray(src)
    dst = np.asarray(dst)

    for k in ("b1", "b2", "b3", "b4", "be1", "be2", "be3", "be4"):
        assert np.abs(p[k]).max() == 0, f"{k} nonzero: unsupported"
    for k in ("g1", "g2", "g3", "g4"):
        assert np.abs(p[k] - 1).max() == 0, f"{k} != 1: unsupported"

    C = np.zeros((N_GRAPHS, GL, GL), np.float32)
    np.add.at(C, (dst // GL, dst % GL, src % GL), 1.0)
    C += np.eye(GL, dtype=np.float32)[None]
    lnC = np.where(C > 0, np.log(np.maximum(C, 1e-30)), np.float32(-1e30)
                   ).astype(np.float32)

    def wcat(W, a_s, a_d, dh, pad_to=None):
        Fin = W.shape[0]
        nh = W.shape[1] // dh
        wes = np.zeros((Fin, 8), np.float64)
        wed = np.zeros((Fin, 8), np.float64)
        for h in range(nh):
            blk = W[:, h * dh:(h + 1) * dh].astype(np.float64)
            wes[:, h] = blk @ a_s[h].astype(np.float64)
            wed[:, h] = blk @ a_d[h].astype(np.float64)
        cat = np.concatenate(
            [W, wes.astype(np.float32), wed.astype(np.float32)], 1)
        if pad_to is not None:
            catp = np.zeros((pad_to, cat.shape[1]), np.float32)
            catp[:Fin] = cat
            cat = catp
        return _b16(cat)

    shared = dict(
        w1=wcat(p["W1"], p["as1"], p["ad1"], 256, pad_to=384),
        w2=wcat(p["W2"], p["as2"], p["ad2"], 256),
        w3=wcat(p["W3"], p["as3"], p["ad3"], 256),
        w4=wcat(p["W4"], p["as4"], p["ad4"], 1024),
        identb=_b16(np.eye(128)),
        identf=np.eye(128, dtype=np.float32),
        wihT2=_b16(np.concatenate([p["Wih_f"].T, p["Wih_b"].T], 1)),
        fc1w=_b16(p["fc1_w"]),
        fc1b=np.ascontiguousarray(p["fc1_b"].reshape(4, 128).T),
        fc2w=_b16(p["fc2_w"]),
        fc2b=p["fc2_b"].reshape(104, 1).copy(),
    )
    bg2 = np.zeros((128, 64), np.float32)
    bg2[:, :32] = (p["bih_f"] + p["bhh_f"]).reshape(32, 128).T
    bg2[:, 32:] = (p["bih_b"] + p["bhh_b"]).reshape(32, 128).T
    shared["bg2"] = bg2
    whh_f = _b16(p["Whh_f"].T)
    whh_b = _b16(p["Whh_b"].T)

    in_maps = []
    for c in range(NCORES):
        nodes = slice(c * NPC, (c + 1) * NPC)
        xT = np.zeros((384, NPC), np.float32)
        xT[:300] = x[nodes].T
        m = dict(shared)
        m["xT"] = _b16(xT)
        m["lnc"] = np.ascontiguousarray(
            lnC[c * GPC:(c + 1) * GPC].transpose(1, 0, 2))
        m["whhT"] = whh_f if c < 4 else whh_b
        in_maps.append(m)
    return in_maps


def _get_nc(dbg=False):
    key = ("nc", dbg)
    if key not in _NC_CACHE:
        _NC_CACHE[key] = _build(dbg=dbg)
    return _NC_CACHE[key]


def kernel(x, src, dst, params, dbg=False, trace=False, tmpdir=None):
    in_maps = _prep_inputs(x, src, dst, params)
    nc = _get_nc(dbg=dbg)
    res = run_bass_kernel_spmd(nc, in_maps, core_ids=list(range(NCORES)),
                               trace=trace, tmpdir=tmpdir)
    out = np.ascontiguousarray(res.results[0]["out"].T.astype(np.float32))
    if dbg or trace:
        return out, res
    return out
